# revision 11
# baseline (speedup 1.0000x reference)
"""Distributed Trainium2 kernel for the diagonal-Rydberg Hamiltonian apply.

Math (n = 22 qubits, dim = 2^22, psi complex as separate real/imag f32):
    out = (rabi/2) * sum_k flip_k(psi) + diag * psi
    diag(b) = sum_k (-detune) * bit_k(b) + sum_{i<j} triu(U,1)[i,j] bit_i(b) bit_j(b)

Distribution: state sharded over 8 cores along the 3 leading qubit axes.
Core d owns amplitudes with global index g = d (top 3 bits). Its output
needs its own shard plus the 3 Hamming-distance-1 partner shards.
All data each core needs is staged in its own DRAM; no collectives.

Per-core layout: local 19 bits -> [128 partitions (bits 12..18), 4096 free
(bits 0..11)]; free axis = 8 chunks of 512 columns (chunk bits 9..11).

Flip-sum strategy (fp8 terms, fp32 PSUM accumulation): 7 DR matmuls per
512-col chunk, each DoubleRow pass summing TWO fp8 k-tiles:
    DR1 [A7|I](own_c, own_c^1)   7 partition flips + chunk-bit-0 flip
    DR2 [I|I](own_c^2, own_c^4)  chunk-bit-1/2 flips
    DR3 [I|I](pb0_c, pb1_c)      partner shards d^1, d^2
    DR4 [I|I](pb2_c, q01_c)      partner d^4 + pair-sum flip0+flip1
    DR5 [I|I](q23_c, j4_c)       pair-sum flip2+flip3 + flip4 copy
    DR6 [I|I](j5_c, j6_c)        flip5/flip6 copies
    DR7 [I|I](j7_c, j8_c)        flip7/flip8 copies
Producers (per 1024-col wave, 2 chunks):
    DVE    q01 = flip0+flip1, q23 = flip2+flip3  (bf16 reads -> fp8)
    ACT    j4, j5, j6 flip copies (bf16 -> fp8)
    DMA    j7, j8 block swaps (fp8 SBUF->SBUF, HWDGE)
    GPSIMD dx = D (.) x_bf16
Diagonal: D built once by a K=9 float32r matmul from host bit tables,
cast to bf16.  Finalize on DVE: out = psum * (rabi/2) + dx (bf16 store).
Host staging is layout-only (dtype casts + shard copies): no host
arithmetic on the state.
"""

import os
import sys

import numpy as np
import ml_dtypes

_REPO = "/opt/trn_rl_repo"
if _REPO not in sys.path:
    sys.path.insert(0, _REPO)

import concourse.mybir as mybir  # noqa: E402
from concourse import bacc  # noqa: E402
from concourse import bass  # noqa: E402
from concourse.tile import TileContext  # noqa: E402
from concourse.bass_utils import run_bass_kernel_spmd  # noqa: E402

N_Q = 22
N_GLOBAL = 3
N_CORES = 8
N_LOCAL = N_Q - N_GLOBAL          # 19
P_BITS = 7                        # partition bits (local bits 12..18)
F_BITS = N_LOCAL - P_BITS         # 12 free bits
P = 1 << P_BITS                   # 128
F = 1 << F_BITS                   # 4096
CHUNK = 512
N_CHUNKS = F // CHUNK             # 8
SHARD = P * F                     # 2^19
WCH = 2                           # chunks per wave
WAVE = WCH * CHUNK                # 1024
N_WAVES = F // WAVE               # 4
PIPE = 3                          # producer lookahead (waves)

BF16 = ml_dtypes.bfloat16
FP8 = ml_dtypes.float8_e4m3

# fp8 SBUF tensor segments (units of F=4096 columns).
SEG_OWN, SEG_PB0, SEG_PB1, SEG_PB2 = 0, 1, 2, 3
SEG_Q01, SEG_Q23, SEG_J4, SEG_J5, SEG_J6, SEG_J7, SEG_J8 = 4, 5, 6, 7, 8, 9, 10
N_DMA_SEG = 4
N_SEG = 11

_cached = {}


def _pair_ap(t, o1, o2, width=CHUNK):
    """Moving AP [128, 2, width] for a DoubleRow pair: k-tile0 at column o1,
    k-tile1 at column o2 of SBUF tile t. o2 > o1 required."""
    base = t[:, o1:o1 + width]
    d = o2 - o1
    assert d > 0
    return bass.AP(tensor=base.tensor, offset=base.offset,
                   ap=[list(base.ap[0]), [d, 2], [1, width]])


def _build_program():
    """Build the (input-independent) Bass program once per process."""
    if "nc" in _cached:
        return _cached["nc"]

    nc = bacc.Bacc("TRN2", num_devices=N_CORES)
    f32, bf16, fp8 = mybir.dt.float32, mybir.dt.bfloat16, mybir.dt.float8e4
    d_dt = bf16
    Alu = mybir.AluOpType

    x8r = nc.dram_tensor("x8r", [P, N_DMA_SEG * F], fp8,
                         kind="ExternalInput")
    x8i = nc.dram_tensor("x8i", [P, N_DMA_SEG * F], fp8,
                         kind="ExternalInput")
    xbr = nc.dram_tensor("xbr", [P, F], bf16, kind="ExternalInput")
    xbi = nc.dram_tensor("xbi", [P, F], bf16, kind="ExternalInput")
    wa7i = nc.dram_tensor("wa7i", [P, 2 * P], fp8, kind="ExternalInput")
    wia7 = nc.dram_tensor("wia7", [P, 2 * P], fp8, kind="ExternalInput")
    wii = nc.dram_tensor("wii", [P, 2 * P], fp8, kind="ExternalInput")
    dlhs = nc.dram_tensor("dlhs", [9, P], d_dt, kind="ExternalInput")
    drhs = nc.dram_tensor("drhs", [9, F], d_dt, kind="ExternalInput")
    rh = nc.dram_tensor("rh", [P, 1], f32, kind="ExternalInput")
    outr = nc.dram_tensor("outr", [P, F], bf16, kind="ExternalOutput")
    outi = nc.dram_tensor("outi", [P, F], bf16, kind="ExternalOutput")

    with TileContext(nc) as tc:
        with (
            tc.tile_pool(name="singles", bufs=1) as singles,
            tc.tile_pool(name="psum", bufs=3, space="PSUM") as psum_pool,
            tc.tile_pool(name="psd", bufs=1, space="PSUM") as psd_pool,
            tc.tile_pool(name="dx", bufs=3) as dx_pool,
            tc.tile_pool(name="osb", bufs=3) as osb_pool,
        ):
            # ---- aux loads (scalar ring, ahead of all bulk traffic) ----
            t_dlhs = singles.tile([9, P], d_dt, tag="dlhs")
            nc.scalar.dma_start(out=t_dlhs[:], in_=dlhs[:])
            t_drhs = singles.tile([9, F], d_dt, tag="drhs")
            nc.scalar.dma_start(out=t_drhs[:], in_=drhs[:])
            t_wa7i = singles.tile([P, 2 * P], fp8, tag="wa7i")
            nc.scalar.dma_start(out=t_wa7i[:], in_=wa7i[:])
            t_wia7 = singles.tile([P, 2 * P], fp8, tag="wia7")
            nc.scalar.dma_start(out=t_wia7[:], in_=wia7[:])
            t_wii = singles.tile([P, 2 * P], fp8, tag="wii")
            nc.scalar.dma_start(out=t_wii[:], in_=wii[:])
            t_rh = singles.tile([P, 1], f32, tag="rh")
            nc.scalar.dma_start(out=t_rh[:], in_=rh[:])

            # ---- bulk loads, wave-major so wave 0 lands first ----
            t_x8, t_xb = {}, {}
            for name in ("r", "i"):
                tb = singles.tile([P, F], bf16, tag=f"xb{name}")
                t_xb[name] = tb
                t8 = singles.tile([P, N_SEG * F], fp8, tag=f"x8{name}")
                t_x8[name] = t8
            # x8 segments on the sync HWDGE ring, xb on the scalar ring
            # (outputs also go on scalar; j7/j8 swaps ride SWDGE/gpsimd).
            for name, db16, d8 in (("r", xbr, x8r), ("i", xbi, x8i)):
                tb, t8 = t_xb[name], t_x8[name]
                for w in range(N_WAVES):
                    ws = slice(w * WAVE, (w + 1) * WAVE)
                    nc.scalar.dma_start(out=tb[:, ws], in_=db16[:, ws])
                    for s in range(N_DMA_SEG):
                        sl = slice(s * F + w * WAVE, s * F + (w + 1) * WAVE)
                        nc.sync.dma_start(out=t8[:, sl], in_=d8[:, sl])

            # ---- diagonal D = dlhs.T @ drhs (K=9), shared by r and i ----
            t_D = singles.tile([P, F], bf16, tag="D")
            for c in range(N_CHUNKS):
                sl = slice(c * CHUNK, (c + 1) * CHUNK)
                pd = psd_pool.tile([P, CHUNK], f32, tag="psd")
                nc.tensor.matmul(pd[:], t_dlhs[:], t_drhs[:, sl],
                                 start=True, stop=True)
                nc.scalar.copy(t_D[:, sl], pd[:])

            # DoubleRow stationary views [K, 2, M]
            v_a7i = t_wa7i[:].rearrange("k (two m) -> k two m", two=2)
            v_ia7 = t_wia7[:].rearrange("k (two m) -> k two m", two=2)
            v_ii = t_wii[:].rearrange("k (two m) -> k two m", two=2)
            DR = mybir.MatmulPerfMode.DoubleRow

            waves = [(name, w) for name in ("r", "i")
                     for w in range(N_WAVES)]
            wave_dx = {}

            def seg(s, c):
                return s * F + c * CHUNK

            def flipwv(xb, j, w):
                """Wave-wide flipped (bit j) view of bf16 tensor xb."""
                b = 1 << j
                v = xb[:, w * WAVE:(w + 1) * WAVE].rearrange(
                    "p (g t b) -> p g t b", t=2, b=b)
                return v[:, :, ::-1, :]

            def swap_ap(t8, off, blk, ngrp):
                """AP [128, ngrp, blk] striding 2*blk between groups."""
                base = t8[:, off:off + blk]
                return bass.AP(tensor=base.tensor, offset=base.offset,
                               ap=[list(base.ap[0]), [2 * blk, ngrp],
                                   [1, blk]])

            def produce(name, w):
                x8 = t_x8[name]
                xb = t_xb[name]
                ws = slice(w * WAVE, (w + 1) * WAVE)
                # DVE: pair-sum flips 0+1 and 2+3 -> fp8 segments
                o_q01 = SEG_Q01 * F + w * WAVE
                nc.vector.tensor_tensor(
                    out=x8[:, o_q01:o_q01 + WAVE],
                    in0=flipwv(xb, 0, w), in1=flipwv(xb, 1, w), op=Alu.add)
                o_q23 = SEG_Q23 * F + w * WAVE
                nc.vector.tensor_tensor(
                    out=x8[:, o_q23:o_q23 + WAVE],
                    in0=flipwv(xb, 2, w), in1=flipwv(xb, 3, w), op=Alu.add)
                # ACT: flip-copies j4..j6 -> fp8 segments
                for j, s in ((4, SEG_J4), (5, SEG_J5), (6, SEG_J6)):
                    o = s * F + w * WAVE
                    nc.scalar.copy(x8[:, o:o + WAVE], flipwv(xb, j, w))
                # DMA (HWDGE rings): j7 = 128-block swap (scalar ring),
                # j8 = 256-block swap (sync ring)
                ow = SEG_OWN * F + w * WAVE
                for blk, sj, eng in ((128, SEG_J7, nc.scalar),
                                     (256, SEG_J8, nc.sync)):
                    oj = sj * F + w * WAVE
                    ngrp = WAVE // (2 * blk)
                    for t in range(2):
                        eng.dma_start(
                            out=swap_ap(x8, oj + t * blk, blk, ngrp),
                            in_=swap_ap(x8, ow + (1 - t) * blk, blk, ngrp))
                # GPSIMD: diag product (bf16 out), wave-wide
                dx = dx_pool.tile([P, WAVE], bf16, tag="dx")
                nc.gpsimd.tensor_tensor(out=dx[:], in0=t_D[:, ws],
                                        in1=xb[:, ws], op=Alu.mult)
                wave_dx[(name, w)] = dx

            def consume(name, w):
                x8 = t_x8[name]
                out_dram = outr if name == "r" else outi
                dx = wave_dx.pop((name, w))
                acc = psum_pool.tile([P, WAVE], f32, tag="acc")
                for ci in range(WCH):
                    c = w * WCH + ci
                    _chunk(x8, acc, ci, c)
                # finalize: out = acc * (rabi/2) + dx (bf16), wave-wide
                osb = osb_pool.tile([P, WAVE], bf16, tag="osb")
                nc.vector.scalar_tensor_tensor(
                    out=osb[:], in0=acc[:], scalar=t_rh[:], in1=dx[:],
                    op0=Alu.mult, op1=Alu.add)
                ws = slice(w * WAVE, (w + 1) * WAVE)
                nc.scalar.dma_start(out=out_dram[:, ws], in_=osb[:])

            def _chunk(x8, acc, ci, c):
                po = acc[:, ci * CHUNK:(ci + 1) * CHUNK]
                c1 = c ^ 1
                if c < c1:
                    nc.tensor.matmul(po, v_a7i,
                                     _pair_ap(x8, c * CHUNK, c1 * CHUNK),
                                     start=True, stop=False, perf_mode=DR)
                else:
                    nc.tensor.matmul(po, v_ia7,
                                     _pair_ap(x8, c1 * CHUNK, c * CHUNK),
                                     start=True, stop=False, perf_mode=DR)
                ca, cb = sorted((c ^ 2, c ^ 4))
                nc.tensor.matmul(po, v_ii,
                                 _pair_ap(x8, ca * CHUNK, cb * CHUNK),
                                 start=False, stop=False, perf_mode=DR)
                for sa, sb, last in ((SEG_PB0, SEG_PB1, False),
                                     (SEG_PB2, SEG_Q01, False),
                                     (SEG_Q23, SEG_J4, False),
                                     (SEG_J5, SEG_J6, False),
                                     (SEG_J7, SEG_J8, True)):
                    nc.tensor.matmul(
                        po, v_ii, _pair_ap(x8, seg(sa, c), seg(sb, c)),
                        start=False, stop=last, perf_mode=DR)

            for wi in range(len(waves) + PIPE):
                if wi < len(waves):
                    produce(*waves[wi])
                if wi >= PIPE:
                    consume(*waves[wi - PIPE])

    nc.finalize()
    _cached["nc"] = nc
    return nc


def _host_tables(U, detune, d):
    """Per-core diagonal tables for the K=9 on-device D matmul."""
    Ut = np.triu(U.astype(np.float64), 1)
    gval = {0: (d >> 2) & 1, 1: (d >> 1) & 1, 2: d & 1}  # qubit -> bit of d
    # linear coefficient for every local qubit (3..21)
    lin = np.zeros(N_Q)
    for q in range(3, N_Q):
        lin[q] = -detune + sum(gval[i] * Ut[i, q] for i in range(3))
    const_d = -detune * sum(gval.values())
    for i in range(3):
        for j in range(i + 1, 3):
            const_d += Ut[i, j] * gval[i] * gval[j]

    hi_q = [9 - m for m in range(P_BITS)]        # partition bit m -> qubit
    lo_q = [21 - r for r in range(F_BITS)]       # free bit r -> qubit

    pidx = np.arange(P)
    B7 = ((pidx[:, None] >> np.arange(P_BITS)[None, :]) & 1).astype(np.float64)
    fidx = np.arange(F)
    B12 = ((fidx[:, None] >> np.arange(F_BITS)[None, :]) & 1).astype(np.float64)

    def pair_coeff(qa, qb):
        return Ut[min(qa, qb), max(qa, qb)]

    M_hh = np.zeros((P_BITS, P_BITS))
    for m in range(P_BITS):
        for m2 in range(m + 1, P_BITS):
            M_hh[m, m2] = pair_coeff(hi_q[m], hi_q[m2])
    M_ll = np.zeros((F_BITS, F_BITS))
    for r in range(F_BITS):
        for r2 in range(r + 1, F_BITS):
            M_ll[r, r2] = pair_coeff(lo_q[r], lo_q[r2])
    cross = np.zeros((P_BITS, F_BITS))
    for m in range(P_BITS):
        for r in range(F_BITS):
            cross[m, r] = pair_coeff(hi_q[m], lo_q[r])

    T1 = const_d + B7 @ np.array([lin[q] for q in hi_q]) \
        + np.einsum("pm,mn,pn->p", B7, M_hh, B7)
    T2 = B12 @ np.array([lin[q] for q in lo_q]) \
        + np.einsum("fm,mn,fn->f", B12, M_ll, B12)

    dlhs = np.vstack([B7.T, np.ones((1, P)), T1[None, :]]).astype(np.float32)
    drhs = np.vstack([cross @ B12.T, T2[None, :],
                      np.ones((1, F))]).astype(np.float32)
    return dlhs, drhs


def kernel(state_real, state_imag, rabi, detune, U, n_qubits, **_unused):
    n = int(n_qubits)
    assert n == N_Q, f"kernel hardcoded for {N_Q} qubits, got {n}"
    sr = np.ascontiguousarray(np.asarray(state_real, np.float32)).reshape(
        N_CORES, SHARD)
    si = np.ascontiguousarray(np.asarray(state_imag, np.float32)).reshape(
        N_CORES, SHARD)
    rabi_f = float(np.asarray(rabi).reshape(-1)[0])
    det_f = float(np.asarray(detune).reshape(-1)[0])
    U_np = np.asarray(U, np.float32)

    sr8 = sr.astype(FP8)
    si8 = si.astype(FP8)
    srb = sr.astype(BF16)
    sib = si.astype(BF16)

    def pack_x8(s8, d):
        return np.concatenate(
            [s8[d].reshape(P, F), s8[d ^ 1].reshape(P, F),
             s8[d ^ 2].reshape(P, F), s8[d ^ 4].reshape(P, F)], axis=1)

    pidx = np.arange(P)
    A7 = (np.bitwise_count(pidx[:, None] ^ pidx[None, :]) == 1).astype(FP8)
    I128 = np.eye(P, dtype=FP8)
    wa7i = np.concatenate([A7, I128], axis=1)
    wia7 = np.concatenate([I128, A7], axis=1)
    wii = np.concatenate([I128, I128], axis=1)
    rh_col = np.full((P, 1), rabi_f * 0.5, np.float32)

    in_maps = []
    for d in range(N_CORES):
        dlhs, drhs = _host_tables(U_np, det_f, d)
        in_maps.append({
            "x8r": pack_x8(sr8, d),
            "x8i": pack_x8(si8, d),
            "xbr": srb[d].reshape(P, F),
            "xbi": sib[d].reshape(P, F),
            "wa7i": wa7i,
            "wia7": wia7,
            "wii": wii,
            "dlhs": dlhs.astype(BF16),
            "drhs": drhs.astype(BF16),
            "rh": rh_col,
        })

    nc = _build_program()
    trace = bool(int(os.environ.get("BASS_KERNEL_TRACE", "0")))
    kwargs = {}
    if trace:
        kwargs["tmpdir"] = os.environ.get("BASS_KERNEL_TRACE_DIR") or None
    res = run_bass_kernel_spmd(
        nc, in_maps, core_ids=list(range(N_CORES)), trace=trace, **kwargs)
    _cached["last_result"] = res

    out = np.empty((2, N_CORES * SHARD), np.float32)
    for d in range(N_CORES):
        out[0, d * SHARD:(d + 1) * SHARD] = res.results[d]["outr"].astype(
            np.float32).reshape(-1)
        out[1, d * SHARD:(d + 1) * SHARD] = res.results[d]["outi"].astype(
            np.float32).reshape(-1)
    return out
